# revision 17
# baseline (speedup 1.0000x reference)
"""Trainium2 Bass kernel for nn_Attention additive-attention pooling.

Reference computation (per batch b):
  img_att = img[b] @ W_img + b_img            (196, 512)
  hid_att = hidden[b] @ W_hid + b_hid         (512,)
  scores  = relu(img_att + hid_att) @ w_att   (196,)   (+b_att, dropped: softmax-invariant)
  attw    = softmax(scores)                   (196,)
  beta    = sigmoid(hidden[b]@w_beta + b_beta)
  context = beta * (attw @ img[b])            (2048,)

Sharding: data-parallel over batch, 16 batches per core on 8 cores.
On-core plan (bf16 matmul path, fp32 softmax/outputs):
  - SWDGE cast-load img batch -> bf16 natural tiles (n-part, c-free), kept resident
  - PE transposes (bf16) -> imgT tiles (c-part, n-free)
  - main matmul: psum[a-part, n] += W[c,a-slice].T @ imgT[c-tile]; ACT relu+bias
  - scores land directly in a (16, N) psum row-per-batch via w_att columns
    masked to the batch's row (compute engines cannot address partition
    bases other than 0/32/64/96, so rows are scattered by the PE itself)
  - batched softmax over (16, N); attw (scaled by beta) transposed via PE
  - pooling accumulates a (16, c-chunk) psum the same masked way
All DMAs are issued on gpsimd (SWDGE): this toolchain's walrus rejects DMA
instructions carrying >1 sync wait; SWDGE DMAs prefixed by a tiny gpsimd
compute op (which absorbs cross-engine waits into the Pool clock) stay at
<=1 wait.
"""

import numpy as np

import concourse.bass as bass
import concourse.bacc as bacc
import concourse.mybir as mybir
import concourse.tile as tile
from concourse.bass import ts
from concourse.bass_utils import run_bass_kernel_spmd
from concourse.masks import make_identity

F32 = mybir.dt.float32
BF16 = mybir.dt.bfloat16
AF = mybir.ActivationFunctionType

B, N, C, H, A = 128, 196, 2048, 512, 512
NCORES = 8
BS = B // NCORES           # 16
N1, N2 = 128, N - 128      # 128 + 68
KC = C // 128              # 16
KA = A // 128              # 4
KH = H // 128              # 4
CCH = 512                  # pooling output chunk
NCC = C // CCH             # 4


def build_nc():
    nc = bacc.Bacc("TRN2", target_bir_lowering=False)
    img = nc.dram_tensor("img", [BS, N, C], F32, kind="ExternalInput").ap()
    hid = nc.dram_tensor("hidden", [BS, H], F32, kind="ExternalInput").ap()
    W_img = nc.dram_tensor("W_img", [C, A], F32, kind="ExternalInput").ap()
    b_img = nc.dram_tensor("b_img", [A], F32, kind="ExternalInput").ap()
    W_hid = nc.dram_tensor("W_hid", [H, A], F32, kind="ExternalInput").ap()
    b_hid = nc.dram_tensor("b_hid", [A], F32, kind="ExternalInput").ap()
    w_att = nc.dram_tensor("w_att", [A], F32, kind="ExternalInput").ap()
    w_beta = nc.dram_tensor("w_beta", [H], F32, kind="ExternalInput").ap()
    b_beta = nc.dram_tensor("b_beta", [1], F32, kind="ExternalInput").ap()
    ctx_out = nc.dram_tensor("context", [BS, C], F32, kind="ExternalOutput").ap()
    attw_out = nc.dram_tensor("attw", [BS, N], F32, kind="ExternalOutput").ap()

    with tile.TileContext(nc) as tc:
        import contextlib
        with contextlib.ExitStack() as ctx:
            const = ctx.enter_context(tc.tile_pool(name="const", bufs=1))
            nat1p = ctx.enter_context(tc.tile_pool(name="nat1", bufs=BS))
            nat2p = ctx.enter_context(tc.tile_pool(name="nat2", bufs=BS))
            tposep = ctx.enter_context(tc.tile_pool(name="tpose", bufs=4))
            rap = ctx.enter_context(tc.tile_pool(name="ra", bufs=3))
            psum = ctx.enter_context(tc.tile_pool(name="psum", bufs=1, space="PSUM"))

            # ---------------- constants / weights ----------------
            # batch-0 image loads issue before anything else on the Pool
            # queue (the ~5us transfer is the startup critical path); the
            # identity construction (Pool compute) runs under that transfer
            nat1_tiles = []
            nat2_tiles = []
            nat1 = nat1p.tile([N1, C], BF16, tag="nat1")
            nat2 = nat2p.tile([N2, C], BF16, tag="nat2")
            nat1_tiles.append(nat1)
            nat2_tiles.append(nat2)
            nc.gpsimd.dma_start(nat1[:], img[0, :N1, :])
            nc.gpsimd.dma_start(nat2[:], img[0, N1:, :])
            ident_bf = const.tile([128, 128], BF16)
            make_identity(nc, ident_bf[:])
            ident_f32 = const.tile([128, 128], F32)
            make_identity(nc, ident_f32[:])
            # W_img in two halves so the first matmuls wait on half the bytes
            Wsb = const.tile([128, KC, A], BF16)       # W_img, c = k*128+p
            Wr = W_img.rearrange("(k p) a -> p k a", p=128)
            nc.gpsimd.dma_start(Wsb[:, :KC // 2, :], Wr[:, :KC // 2, :])
            nc.gpsimd.dma_start(Wsb[:, KC // 2:, :], Wr[:, KC // 2:, :])
            hid_sb = const.tile([BS, H], F32)
            nc.sync.dma_start(hid_sb[:], hid)
            Whid = const.tile([128, KH, A], F32)       # W_hid, h = k*128+p
            nc.sync.dma_start(Whid[:], W_hid.rearrange("(k p) a -> p k a", p=128))
            bimgT = const.tile([128, KA], F32)
            nc.sync.dma_start(bimgT[:], b_img.rearrange("(k p) -> p k", p=128))
            bhidT = const.tile([128, KA], F32)
            nc.sync.dma_start(bhidT[:], b_hid.rearrange("(k p) -> p k", p=128))
            wattT = const.tile([128, KA], F32)
            nc.sync.dma_start(wattT[:], w_att.rearrange("(k p) -> p k", p=128))
            wbetaT = const.tile([128, KH], F32)
            nc.sync.dma_start(wbetaT[:], w_beta.rearrange("(k p) -> p k", p=128))
            bbeta = const.tile([1, 1], F32)
            nc.sync.dma_start(bbeta[:], b_beta[None, :])

            # w_att as bf16, and per-batch diagonal masks:
            # wmask[p, m, r, b] = w_att[m*128+p] * (r == b)
            wattT_bf = const.tile([128, KA], BF16)
            nc.vector.tensor_copy(wattT_bf[:], wattT[:])
            wmask = const.tile([128, KA, BS, BS], BF16)
            nc.vector.memset(wmask[:], 0.0)
            for b in range(BS):
                nc.vector.tensor_copy(wmask[:, :, b, b], wattT_bf[:])

            # pooling mask shells zeroed up front, off the softmax chain
            amask1 = const.tile([N1, BS, BS], BF16)
            nc.vector.memset(amask1[:], 0.0)
            amask2 = const.tile([N2, BS, BS], BF16)
            nc.vector.memset(amask2[:], 0.0)

            # bcomb[p, m] = b_img + b_hid (both indexed by a = m*128+p)
            bcomb = const.tile([128, KA], F32)
            nc.vector.tensor_add(bcomb[:], bimgT[:], bhidT[:])

            # ---------------- hidden-side (once) ----------------
            # hiddenT[p, k, b] = hidden[b, k*128+p] via PE transposes
            hiddenT = const.tile([128, KH, BS], F32)
            for k in range(KH):
                ps_t = psum.tile([128, BS], F32, tag="mm", bufs=2)
                nc.tensor.transpose(ps_t[:], hid_sb[:, ts(k, 128)], ident_f32[:BS, :BS])
                nc.scalar.copy(hiddenT[:, k, :], ps_t[:])

            # hid_attT[p, m, b] = (hidden @ W_hid).T ; bias_all = hid_attT + b_img + b_hid
            bias_all = const.tile([128, KA, BS], F32)
            for m in range(KA):
                ps_m = psum.tile([128, BS], F32, tag="mm", bufs=2)
                for k in range(KH):
                    nc.tensor.matmul(ps_m[:], Whid[:, k, ts(m, 128)], hiddenT[:, k, :],
                                     start=(k == 0), stop=(k == KH - 1))
                nc.vector.tensor_add(
                    bias_all[:, m, :], ps_m[:],
                    bcomb[:, m, None].to_broadcast((128, BS)))

            # beta = sigmoid(hidden @ w_beta + b_beta): row (1, BS), then a
            # PE transpose to the (BS, 1) column used as per-partition scalar
            ps_b = psum.tile([1, BS], F32, tag="ps_s", bufs=1)
            for k in range(KH):
                nc.tensor.matmul(ps_b[:], wbetaT[:, k, None], hiddenT[:, k, :],
                                 start=(k == 0), stop=(k == KH - 1))
            beta_row = const.tile([1, BS], F32)
            nc.scalar.activation(beta_row[:], ps_b[:], AF.Sigmoid, bias=bbeta[0:1, 0:1])
            ps_bc = psum.tile([BS, 1], F32, tag="ps_s", bufs=1)
            nc.tensor.transpose(ps_bc[:], beta_row[:], ident_f32[:1, :1])
            beta_col = const.tile([BS, 1], F32)
            nc.scalar.copy(beta_col[:], ps_bc[:])

            # ---------------- phase 1: per-batch scores ----------------
            # scores accumulate into one (BS, N) psum: batch b's 4 matmuls use
            # the w_att columns masked to row b
            ps_scores = psum.tile([BS, N], F32, tag="scores", bufs=1)
            # issue the remaining cast-loads up front (batch 0 was issued
            # before the weights): no deps, dedicated slots
            for b in range(1, BS):
                nat1 = nat1p.tile([N1, C], BF16, tag="nat1")
                nat2 = nat2p.tile([N2, C], BF16, tag="nat2")
                nat1_tiles.append(nat1)
                nat2_tiles.append(nat2)
                nc.gpsimd.dma_start(nat1[:], img[b, :N1, :])   # fp32 -> bf16 cast DMA
                nc.gpsimd.dma_start(nat2[:], img[b, N1:, :])
            for b in range(BS):
                nat1 = nat1_tiles[b]
                nat2 = nat2_tiles[b]
                # transpose into imgT[p, k, n]: imgT[p,k,n] = img[b, n, k*128+p]
                imgT = tposep.tile([128, KC, N], BF16, tag="imgT")
                for k in range(KC):
                    # both n-tiles transpose into one psum tile -> one copy
                    ps_t = psum.tile([128, N], BF16, tag="tr", bufs=4)
                    nc.tensor.transpose(ps_t[:, :N1], nat1[:, ts(k, 128)], ident_bf[:N1, :N1])
                    nc.tensor.transpose(ps_t[:, N1:], nat2[:, ts(k, 128)], ident_bf[:N2, :N2])
                    if k % 2 == 0:
                        nc.scalar.copy(imgT[:, k, :], ps_t[:])
                    else:
                        nc.vector.tensor_copy(imgT[:, k, :], ps_t[:])

                raT = rap.tile([128, KA, N], BF16, tag="raT")
                for m in range(KA):
                    ps = psum.tile([128, N], F32, tag="mm", bufs=2)
                    for k in range(KC):
                        nc.tensor.matmul(ps[:], Wsb[:, k, ts(m, 128)], imgT[:, k, :],
                                         start=(k == 0), stop=(k == KC - 1))
                    # relu(img_att + b_img + hid_att[b]) -> bf16
                    nc.scalar.activation(raT[:, m, :], ps[:], AF.Relu,
                                         bias=bias_all[:, m, b, None])

                for m in range(KA):
                    nc.tensor.matmul(ps_scores[:], wmask[:, m, :, b], raT[:, m, :],
                                     start=(b == 0 and m == 0),
                                     stop=(b == BS - 1 and m == KA - 1),
                                     skip_group_check=True)

            # ---------------- phase 2: softmax + attw transpose ----------------
            negmax = const.tile([BS, 1], F32)
            nc.vector.tensor_reduce(negmax[:], ps_scores[:], axis=mybir.AxisListType.X,
                                    op=mybir.AluOpType.max, negate=True)
            attw_all = const.tile([BS, N], F32)
            ssum = const.tile([BS, 1], F32)
            nc.scalar.activation(attw_all[:], ps_scores[:], AF.Exp,
                                 bias=negmax[:, 0:1], accum_out=ssum[:, 0:1])
            rinv = const.tile([BS, 1], F32)
            nc.vector.reciprocal(rinv[:], ssum[:])
            nc.vector.tensor_scalar_mul(attw_all[:], attw_all[:], rinv[:, 0:1])

            # beta-scaled bf16 copy feeds the pooling path
            attw_bf = const.tile([BS, N], BF16)
            nc.vector.tensor_scalar_mul(attw_bf[:], attw_all[:], beta_col[:, 0:1])
            # mask diagonals copied straight from the transpose psums,
            # split across DVE and ACT to shorten the serial chain
            ps_a1 = psum.tile([N1, BS], BF16, tag="tr", bufs=4)
            nc.tensor.transpose(ps_a1[:], attw_bf[:, :N1], ident_bf[:BS, :BS])
            ps_a2 = psum.tile([N2, BS], BF16, tag="tr", bufs=4)
            nc.tensor.transpose(ps_a2[:], attw_bf[:, N1:], ident_bf[:BS, :BS])
            for b in range(BS):
                if b % 2 == 0:
                    nc.vector.tensor_copy(amask1[:, b, b, None], ps_a1[:, b, None])
                    nc.scalar.copy(amask2[:, b, b, None], ps_a2[:, b, None])
                else:
                    nc.scalar.copy(amask1[:, b, b, None], ps_a1[:, b, None])
                    nc.vector.tensor_copy(amask2[:, b, b, None], ps_a2[:, b, None])

            # ---------------- phase 3: pooling ----------------
            ctx_all = const.tile([BS, C], F32)
            for cc in range(NCC):
                ps_p = psum.tile([BS, CCH], F32, tag="mm", bufs=2)
                for b in range(BS):
                    nc.tensor.matmul(ps_p[:], amask1[:, :, b],
                                     nat1_tiles[b][:, ts(cc, CCH)],
                                     start=(b == 0), stop=False,
                                     skip_group_check=True)
                    nc.tensor.matmul(ps_p[:], amask2[:, :, b],
                                     nat2_tiles[b][:, ts(cc, CCH)],
                                     start=False, stop=(b == BS - 1),
                                     skip_group_check=True)
                if cc % 2 == 0:
                    nc.vector.tensor_copy(ctx_all[:, ts(cc, CCH)], ps_p[:])
                else:
                    nc.scalar.copy(ctx_all[:, ts(cc, CCH)], ps_p[:])

            # ---------------- outputs ----------------
            # HWDGE outputs: Bacc's event-semaphore legalization handles the
            # multi-wait sync, and SP issue avoids the ~1us SWDGE Q7 emission
            nc.sync.dma_start(attw_out, attw_all[:])
            # two half-width ctx DMAs: the first fires before the last
            # pooling chunks finish
            nc.sync.dma_start(ctx_out[:, :C // 2], ctx_all[:, :C // 2])
            nc.sync.dma_start(ctx_out[:, C // 2:], ctx_all[:, C // 2:])
    if not nc.is_finalized():
        nc.finalize()
    return nc


_NC = None


def _get_nc():
    global _NC
    if _NC is None:
        _NC = build_nc()
    return _NC


def kernel(img_embedding, hidden, W_img, b_img, W_hid, b_hid,
           w_att, b_att, w_beta, b_beta):
    img_embedding = np.ascontiguousarray(img_embedding, dtype=np.float32)
    hidden = np.ascontiguousarray(hidden, dtype=np.float32)
    shared = {
        "W_img": np.ascontiguousarray(W_img, dtype=np.float32),
        "b_img": np.ascontiguousarray(b_img, dtype=np.float32),
        "W_hid": np.ascontiguousarray(W_hid, dtype=np.float32),
        "b_hid": np.ascontiguousarray(b_hid, dtype=np.float32),
        "w_att": np.ascontiguousarray(w_att, dtype=np.float32),
        "w_beta": np.ascontiguousarray(w_beta, dtype=np.float32),
        "b_beta": np.ascontiguousarray(np.reshape(b_beta, (1,)), dtype=np.float32),
    }
    in_maps = []
    for c in range(NCORES):
        sl = slice(c * BS, (c + 1) * BS)
        in_maps.append({
            "img": img_embedding[sl],
            "hidden": hidden[sl],
            **shared,
        })
    res = run_bass_kernel_spmd(_get_nc(), in_maps, core_ids=list(range(NCORES)))
    ctx = np.concatenate([r["context"] for r in res.results], axis=0)
    attw = np.concatenate([r["attw"] for r in res.results], axis=0)
    return ctx, attw
